# revision 25
# baseline (speedup 1.0000x reference)
"""MoCo forward + queue update kernel for Trainium2 (8 NeuronCores, SPMD).

Problem (nn_MoCo): given q,k [512,128] f32, targets [512] i32 (7 classes),
queues [7,128,65536] f32, ptrs [7] i32, compute
  l_pos[n]  = q[n].k[n]
  l_neg[n,:] = q[n] @ queues[targets[n]]
  logits    = concat([l_pos, l_neg], 1) / 0.07
  labels    = zeros
  new_queues = queues with column (t, ptrs[t]+rank) <- k[n] for each sample
  new_ptrs  = (ptrs + counts) % K

Sharding: split K=65536 into 8 shards of 8192 columns, one per core.  Each
core loads its shard of every queue bank once into SBUF, uses it as matmul
rhs (samples grouped by class so each bank is multiplied only by its own
class's queries -- 7x less FLOPs than the dense all-classes form), applies
the queue-update in SBUF, and stores the updated shard + its l_neg columns.
The per-class scatter positions are a consecutive run (ring-buffer
enqueue), so the update is a single dynamic-offset windowed copy per run,
padded with original queue columns (no-op writes) to keep the SPMD program
identical across cores.  No collectives; host concatenates the shards.
"""

import numpy as np

import concourse.bass as bass
import concourse.mybir as mybir
import concourse.tile as tile
from concourse import bacc
from concourse.bass import ds
from concourse.bass_utils import run_bass_kernel_spmd

N, C, K, T = 512, 128, 65536, 7
TEMP = 0.07
NCORES = 8
SH = K // NCORES  # 8192 columns per core
CHUNK = 512  # matmul free dim (fp32 max)
HB = 4096  # pipeline tile width (half bank)
NH = SH // HB  # tiles per bank
RUNS = 1  # dynamic-update run slots per (core, bank, half)
USE_F16 = False  # cast matmul operands to fp16 (4x faster PE, ~1e-3 relerr)

f32 = mybir.dt.float32
f32r = mybir.dt.float32r
i32 = mybir.dt.int32

_cache: dict = {}
DEBUG_DISABLE: set = set()  # dev-only: {"dyn", "mm", "lpos", "nptr"}


def _build(counts, S, rep=1):
    """Build the per-core Bass program (shared SPMD across all 8 cores).

    counts: per-class sample counts (compile-time constants).
    S: width of each dynamic-update window (max run length).
    rep: repeat the main pipeline (timing-harness use only).
    """
    counts = [int(x) for x in counts]
    row_start = [0] * (T + 1)
    for t in range(T):
        row_start[t + 1] = row_start[t] + counts[t]
    inv_temp = float(1.0 / np.float32(TEMP))

    nc = bacc.Bacc(None, target_bir_lowering=False)
    qT = nc.dram_tensor("qt", [C, N], f32, kind="ExternalInput")
    qrow = nc.dram_tensor("qrow", [N, C], f32, kind="ExternalInput")
    krow = nc.dram_tensor("krow", [N, C], f32, kind="ExternalInput")
    qsh = nc.dram_tensor("qsh", [T, C, SH], f32, kind="ExternalInput")
    ks = nc.dram_tensor("ks", [C, T * S], f32, kind="ExternalInput")
    offt = nc.dram_tensor("offt", [1, T], i32, kind="ExternalInput")
    ptrs = nc.dram_tensor("ptrs", [1, T], i32, kind="ExternalInput")
    cnts = nc.dram_tensor("cnts", [1, T], i32, kind="ExternalInput")
    lneg = nc.dram_tensor("lneg", [N, SH], f32, kind="ExternalOutput")
    newq = nc.dram_tensor("newq", [T, C, SH], f32, kind="ExternalOutput")
    lpos = nc.dram_tensor("lpos", [C, N // C], f32, kind="ExternalOutput")
    nptr = nc.dram_tensor("nptr", [1, T], i32, kind="ExternalOutput")

    with tile.TileContext(nc) as tc:
        with (
            tc.tile_pool(name="singles", bufs=1) as singles,
            tc.tile_pool(name="banks", bufs=6) as banks,
            tc.tile_pool(name="lnegs", bufs=4) as lnegs,
            tc.tile_pool(name="psum", bufs=8, space="PSUM") as psum,
        ):
            # --- queries, pre-scaled by 1/TEMP so PSUM holds scaled logits
            mmdt = mybir.dt.float16 if USE_F16 else f32
            qt_raw = singles.tile([C, N], f32)
            nc.sync.dma_start(out=qt_raw[:], in_=qT[:, :])
            qts = singles.tile([C, N], mmdt)
            nc.vector.tensor_scalar_mul(qts[:], qt_raw[:], inv_temp)

            # --- l_pos: per-sample q.k via elementwise mul + free-dim reduce
            B = N // C  # 4 row-blocks of 128
            if "lpos" not in DEBUG_DISABLE:
                qp = singles.tile([C, B, C], f32)
                nc.sync.dma_start(
                    out=qp[:], in_=qrow[:, :].rearrange("(b p) c -> p b c", p=C)
                )
                kp = singles.tile([C, B, C], f32)
                nc.sync.dma_start(
                    out=kp[:], in_=krow[:, :].rearrange("(b p) c -> p b c", p=C)
                )
                prod = singles.tile([C, B, C], f32)
                nc.vector.tensor_mul(prod[:], qp[:], kp[:])
                lp = singles.tile([C, B], f32)
                nc.vector.reduce_sum(lp[:], prod[:], axis=mybir.AxisListType.X)
                lps = singles.tile([C, B], f32)
                nc.scalar.mul(lps[:], lp[:], inv_temp)
                nc.sync.dma_start(out=lpos[:, :], in_=lps[:])

            # --- new_ptrs = (ptrs + counts) % K  (sum < 2K so mod = cond-sub)
            if "nptr" not in DEBUG_DISABLE:
                pt = singles.tile([1, T], i32)
                nc.sync.dma_start(out=pt[:], in_=ptrs[:, :])
                ct = singles.tile([1, T], i32)
                nc.sync.dma_start(out=ct[:], in_=cnts[:, :])
                s1 = singles.tile([1, T], i32)
                nc.vector.tensor_add(s1[:], pt[:], ct[:])
                mge = singles.tile([1, T], i32)
                nc.vector.tensor_scalar(
                    mge[:], s1[:], K, None, op0=mybir.AluOpType.is_ge
                )
                mk = singles.tile([1, T], i32)
                nc.vector.tensor_scalar_mul(mk[:], mge[:], K)
                s2 = singles.tile([1, T], i32)
                nc.vector.tensor_tensor(
                    s2[:], s1[:], mk[:], op=mybir.AluOpType.subtract
                )
                nc.sync.dma_start(out=nptr[:, :], in_=s2[:])

            # --- scatter sources: per bank a [C, S] column window holding
            # the post-update content, written straight to DRAM after the
            # bulk copy (so the bulk store never waits on the update)
            kst = singles.tile([C, T * S], f32)
            nc.sync.dma_start(out=kst[:], in_=ks[:, :])
            oft = singles.tile([1, T], i32)
            nc.sync.dma_start(out=oft[:], in_=offt[:, :])

            # --- main pipeline: per (bank, half-bank) tile
            for t in [t for _ in range(rep) for t in range(T)]:
                bulk_outs = []
                for h in range(NH):
                    bank = banks.tile([C, HB], f32)
                    nc.sync.dma_start(
                        out=bank[:], in_=qsh[t, :, h * HB : (h + 1) * HB]
                    )
                    if USE_F16:
                        bank_mm = banks.tile([C, HB], mmdt, tag="bank_mm")
                        nc.gpsimd.tensor_copy(out=bank_mm[:], in_=bank[:])
                    else:
                        bank_mm = bank
                    # l_neg rows for class t (row groups of <=128)
                    m0 = 0
                    if "mm" in DEBUG_DISABLE:
                        m0 = counts[t]
                    while m0 < counts[t]:
                        M = min(128, counts[t] - m0)
                        r0 = row_start[t] + m0
                        lnt = lnegs.tile([C, HB], f32)
                        for j in range(HB // CHUNK):
                            ps = psum.tile([128, CHUNK], f32, space="PSUM")
                            nc.tensor.matmul(
                                ps[:M],
                                lhsT=qts[:, r0 : r0 + M],
                                rhs=bank_mm[:, j * CHUNK : (j + 1) * CHUNK],
                                start=True,
                                stop=True,
                            )
                            # split PSUM->SBUF copies across ACT and DVE
                            if j % 2 == 0:
                                nc.scalar.copy(
                                    lnt[:M, j * CHUNK : (j + 1) * CHUNK], ps[:M]
                                )
                            else:
                                nc.vector.tensor_copy(
                                    out=lnt[:M, j * CHUNK : (j + 1) * CHUNK],
                                    in_=ps[:M],
                                )
                        nc.gpsimd.dma_start(
                            out=lneg[r0 : r0 + M, h * HB : (h + 1) * HB],
                            in_=lnt[:M, :],
                        )
                        m0 += M
                    bulk_outs.append(
                        nc.gpsimd.dma_start(
                            out=newq[t, :, h * HB : (h + 1) * HB], in_=bank[:]
                        )
                    )
                # queue update: one dynamic-offset window DMA per bank,
                # overwriting the just-written bulk copy in DRAM.  Cores
                # with no update in this bank write original columns back
                # (the window always holds post-update content).
                if "dyn" not in DEBUG_DISABLE:
                    off = nc.values_load(
                        oft[0:1, t : t + 1],
                        min_val=0,
                        max_val=SH - S,
                        skip_runtime_bounds_check=True,
                    )
                    wdma = nc.gpsimd.dma_start(
                        out=newq[t, :, ds(off, S)],
                        in_=kst[:, t * S : (t + 1) * S],
                    )
                    for b in bulk_outs:
                        tile.add_dep_helper(
                            wdma.ins, b.ins, reason="queue window after bulk"
                        )
    nc.compile()
    return nc


def _prepare(q, k, targets, queues, ptrs):
    """Host-side input prep: class grouping + scatter run construction."""
    perm = np.argsort(targets, kind="stable")
    tsorted = targets[perm]
    counts = np.bincount(targets, minlength=T)
    row_start = np.zeros(T + 1, np.int64)
    row_start[1:] = np.cumsum(counts)

    # positions: for class t the sorted samples go to ptrs[t], ptrs[t]+1, ...
    pos_sorted = np.empty(N, np.int64)
    for t in range(T):
        a, b = int(row_start[t]), int(row_start[t + 1])
        pos_sorted[a:b] = (int(ptrs[t]) + np.arange(b - a)) % K

    k_sorted = np.ascontiguousarray(k[perm])  # [N, C]

    # per (core, bank): the contiguous run of updates landing in this
    # core's shard.  (j0 = column offset within the shard, L, u0)
    runs = [[[] for _ in range(T)] for _ in range(NCORES)]
    for t in range(T):
        a, b = int(row_start[t]), int(row_start[t + 1])
        u = a
        while u < b:
            p = int(pos_sorted[u])
            c = p // SH
            # run end: consecutive positions within the same shard
            end = u + 1
            while (
                end < b
                and int(pos_sorted[end]) == int(pos_sorted[end - 1]) + 1
                and int(pos_sorted[end]) // SH == c
            ):
                end += 1
            runs[c][t].append((p - c * SH, end - u, u))
            u = end

    max_runs = max(len(runs[c][t]) for c in range(NCORES) for t in range(T))
    assert max_runs <= 1, f"expected <=1 run per (core, bank), got {max_runs}"
    S = max(
        [L for c in range(NCORES) for t in range(T) for (_, L, _) in runs[c][t]]
        + [1]
    )
    S = min(max(S, 16), SH)  # small floor reduces recompiles across inputs

    ks_host = np.empty((NCORES, C, T * S), np.float32)
    off_host = np.zeros((NCORES, 1, T), np.int32)
    for c in range(NCORES):
        qslab = queues[:, :, c * SH : (c + 1) * SH]
        for t in range(T):
            if runs[c][t]:
                j0, L, u0 = runs[c][t][0]
                off = max(0, min(j0, SH - S))
            else:
                j0, L, u0, off = 0, 0, 0, 0  # no-op window
            # window = post-update content of shard columns [off, off+S)
            w = np.array(qslab[t, :, off : off + S])
            if L:
                w[:, j0 - off : j0 - off + L] = k_sorted[u0 : u0 + L].T
            ks_host[c, :, t * S : (t + 1) * S] = w
            off_host[c, 0, t] = off
    return perm, counts, S, ks_host, off_host


def kernel(**inputs):
    q = np.ascontiguousarray(np.asarray(inputs["q"], dtype=np.float32))
    k = np.ascontiguousarray(np.asarray(inputs["k"], dtype=np.float32))
    targets = np.asarray(inputs["targets"], dtype=np.int32)
    queues = np.ascontiguousarray(np.asarray(inputs["queues"], dtype=np.float32))
    ptrs = np.asarray(inputs["ptrs"], dtype=np.int32)

    perm, counts, S, ks_host, off_host = _prepare(q, k, targets, queues, ptrs)

    key = (tuple(int(x) for x in counts), S)
    if key not in _cache:
        _cache[key] = _build(counts, S)
    nc = _cache[key]

    qT_sorted = np.ascontiguousarray(q[perm].T)
    cnts_in = counts.astype(np.int32).reshape(1, T)
    ptrs_in = np.ascontiguousarray(ptrs.reshape(1, T))

    in_maps = []
    for c in range(NCORES):
        in_maps.append(
            {
                "qt": qT_sorted,
                "qrow": q,
                "krow": k,
                "qsh": np.ascontiguousarray(queues[:, :, c * SH : (c + 1) * SH]),
                "ks": np.ascontiguousarray(ks_host[c]),
                "offt": off_host[c],
                "ptrs": ptrs_in,
                "cnts": cnts_in,
            }
        )
    res = run_bass_kernel_spmd(nc, in_maps, core_ids=list(range(NCORES)))
    results = res.results

    logits = np.empty((N, K + 1), np.float32)
    logits[:, 0] = results[0]["lpos"].T.reshape(N)
    for c in range(NCORES):
        logits[perm, 1 + c * SH : 1 + (c + 1) * SH] = results[c]["lneg"]
    new_queues = np.empty((T, C, K), np.float32)
    for c in range(NCORES):
        new_queues[:, :, c * SH : (c + 1) * SH] = results[c]["newq"]
    labels = np.zeros(N, np.int32)
    new_ptrs = results[0]["nptr"].reshape(T).astype(np.int32)
    return logits, labels, new_queues, new_ptrs


# revision 38
# speedup vs baseline: 1.1153x; 1.1153x over previous
"""MoCo forward + queue update kernel for Trainium2 (8 NeuronCores, SPMD).

Problem (nn_MoCo): given q,k [512,128] f32, targets [512] i32 (7 classes),
queues [7,128,65536] f32, ptrs [7] i32, compute
  l_pos[n]  = q[n].k[n]
  l_neg[n,:] = q[n] @ queues[targets[n]]
  logits    = concat([l_pos, l_neg], 1) / 0.07
  labels    = zeros
  new_queues = queues with column (t, ptrs[t]+rank) <- k[n] for each sample
  new_ptrs  = (ptrs + counts) % K

Sharding: split K=65536 into 8 shards of 8192 columns, one per core.  Each
core loads its shard of every queue bank once into SBUF, uses it as matmul
rhs (samples grouped by class so each bank is multiplied only by its own
class's queries -- 7x less FLOPs than the dense all-classes form), applies
the queue-update in SBUF, and stores the updated shard + its l_neg columns.
The per-class scatter positions are a consecutive run (ring-buffer
enqueue), so the update is a single dynamic-offset windowed copy per run,
padded with original queue columns (no-op writes) to keep the SPMD program
identical across cores.  No collectives; host concatenates the shards.
"""

import numpy as np

import concourse.bass as bass
import concourse.mybir as mybir
import concourse.tile as tile
from concourse import bacc
from concourse.bass import ds
from concourse.bass_utils import run_bass_kernel_spmd

N, C, K, T = 512, 128, 65536, 7
TEMP = 0.07
NCORES = 8
SH = K // NCORES  # 8192 columns per core
CHUNK = 512  # matmul free dim (fp32 max)
HB = 4096  # pipeline tile width (half bank)
NH = SH // HB  # tiles per bank
RUNS = 1  # dynamic-update run slots per (core, bank, half)
USE_F16 = False  # cast matmul operands to fp16 (4x faster PE, ~1e-3 relerr)

f32 = mybir.dt.float32
f32r = mybir.dt.float32r
i32 = mybir.dt.int32

_cache: dict = {}
DEBUG_DISABLE: set = set()  # dev-only: {"dyn", "mm", "lpos", "nptr"}


def _build(counts, S, rep=1):
    """Build the per-core Bass program (shared SPMD across all 8 cores).

    counts: per-class sample counts (compile-time constants).
    S: width of each dynamic-update window (max run length).
    rep: repeat the main pipeline (timing-harness use only).
    """
    counts = [int(x) for x in counts]
    row_start = [0] * (T + 1)
    for t in range(T):
        row_start[t + 1] = row_start[t] + counts[t]
    inv_temp = float(1.0 / np.float32(TEMP))

    nc = bacc.Bacc(None, target_bir_lowering=False)
    qT = nc.dram_tensor("qt", [C, N], f32, kind="ExternalInput")
    qrow = nc.dram_tensor("qrow", [N, C], f32, kind="ExternalInput")
    krow = nc.dram_tensor("krow", [N, C], f32, kind="ExternalInput")
    qsh = nc.dram_tensor("qsh", [T, C, SH], f32, kind="ExternalInput")
    ks = nc.dram_tensor("ks", [C, T * S], f32, kind="ExternalInput")
    offt = nc.dram_tensor("offt", [1, T], i32, kind="ExternalInput")
    ptrs = nc.dram_tensor("ptrs", [1, T], i32, kind="ExternalInput")
    cnts = nc.dram_tensor("cnts", [1, T], i32, kind="ExternalInput")
    lneg = nc.dram_tensor("lneg", [N, SH], f32, kind="ExternalOutput")
    newq = nc.dram_tensor("newq", [T, C, SH], f32, kind="ExternalOutput")
    lpos = nc.dram_tensor("lpos", [C, N // C], f32, kind="ExternalOutput")
    nptr = nc.dram_tensor("nptr", [1, T], i32, kind="ExternalOutput")

    with tile.TileContext(nc) as tc:
        with (
            tc.tile_pool(name="singles", bufs=1) as singles,
            tc.tile_pool(name="banks", bufs=6) as banks,
            tc.tile_pool(name="lnegs", bufs=4) as lnegs,
            tc.tile_pool(name="psum", bufs=8, space="PSUM") as psum,
        ):
            # --- queries, pre-scaled by 1/TEMP so PSUM holds scaled logits
            mmdt = mybir.dt.float16 if USE_F16 else f32
            qt_raw = singles.tile([C, N], f32)
            nc.sync.dma_start(out=qt_raw[:], in_=qT[:, :])
            qts = singles.tile([C, N], mmdt)
            nc.vector.tensor_scalar_mul(qts[:], qt_raw[:], inv_temp)

            # --- l_pos: per-sample q.k via elementwise mul + free-dim reduce
            B = N // C  # 4 row-blocks of 128
            if "lpos" not in DEBUG_DISABLE:
                qp = singles.tile([C, B, C], f32)
                nc.sync.dma_start(
                    out=qp[:], in_=qrow[:, :].rearrange("(b p) c -> p b c", p=C)
                )
                kp = singles.tile([C, B, C], f32)
                nc.sync.dma_start(
                    out=kp[:], in_=krow[:, :].rearrange("(b p) c -> p b c", p=C)
                )
                prod = singles.tile([C, B, C], f32)
                nc.vector.tensor_mul(prod[:], qp[:], kp[:])
                lp = singles.tile([C, B], f32)
                nc.vector.reduce_sum(lp[:], prod[:], axis=mybir.AxisListType.X)
                lps = singles.tile([C, B], f32)
                nc.scalar.mul(lps[:], lp[:], inv_temp)
                nc.sync.dma_start(out=lpos[:, :], in_=lps[:])

            # --- new_ptrs = (ptrs + counts) % K  (sum < 2K so mod = cond-sub)
            if "nptr" not in DEBUG_DISABLE:
                pt = singles.tile([1, T], i32)
                nc.sync.dma_start(out=pt[:], in_=ptrs[:, :])
                ct = singles.tile([1, T], i32)
                nc.sync.dma_start(out=ct[:], in_=cnts[:, :])
                s1 = singles.tile([1, T], i32)
                nc.vector.tensor_add(s1[:], pt[:], ct[:])
                mge = singles.tile([1, T], i32)
                nc.vector.tensor_scalar(
                    mge[:], s1[:], K, None, op0=mybir.AluOpType.is_ge
                )
                mk = singles.tile([1, T], i32)
                nc.vector.tensor_scalar_mul(mk[:], mge[:], K)
                s2 = singles.tile([1, T], i32)
                nc.vector.tensor_tensor(
                    s2[:], s1[:], mk[:], op=mybir.AluOpType.subtract
                )
                nc.sync.dma_start(out=nptr[:, :], in_=s2[:])

            # --- scatter sources: per bank a [C, S] column window holding
            # the post-update content, written straight to DRAM after the
            # bulk copy (so the bulk store never waits on the update)
            kst = singles.tile([C, T * S], f32)
            nc.sync.dma_start(out=kst[:], in_=ks[:, :])
            oft = singles.tile([1, T], i32)
            nc.sync.dma_start(out=oft[:], in_=offt[:, :])

            # --- main pipeline: per (bank, half-bank) tile
            last_bulk_outs = {t: [] for t in range(T)}  # final-rep bulk outs
            for t in [t for _ in range(rep) for t in range(T)]:
                for h in range(NH):
                    bank = banks.tile([C, HB], f32)
                    nc.sync.dma_start(
                        out=bank[:], in_=qsh[t, :, h * HB : (h + 1) * HB]
                    )
                    if USE_F16:
                        bank_mm = banks.tile([C, HB], mmdt, tag="bank_mm")
                        nc.gpsimd.tensor_copy(out=bank_mm[:], in_=bank[:])
                    else:
                        bank_mm = bank
                    # l_neg rows for class t (row groups of <=128)
                    m0 = 0
                    if "mm" in DEBUG_DISABLE:
                        m0 = counts[t]
                    while m0 < counts[t]:
                        M = min(128, counts[t] - m0)
                        r0 = row_start[t] + m0
                        lnt = lnegs.tile([C, HB], f32)
                        for j in range(HB // CHUNK):
                            ps = psum.tile([128, CHUNK], f32, space="PSUM")
                            nc.tensor.matmul(
                                ps[:M],
                                lhsT=qts[:, r0 : r0 + M],
                                rhs=bank_mm[:, j * CHUNK : (j + 1) * CHUNK],
                                start=True,
                                stop=True,
                            )
                            # split PSUM->SBUF copies across ACT and DVE
                            if j % 2 == 0:
                                nc.scalar.copy(
                                    lnt[:M, j * CHUNK : (j + 1) * CHUNK], ps[:M]
                                )
                            else:
                                nc.vector.tensor_copy(
                                    out=lnt[:M, j * CHUNK : (j + 1) * CHUNK],
                                    in_=ps[:M],
                                )
                        nc.gpsimd.dma_start(
                            out=lneg[r0 : r0 + M, h * HB : (h + 1) * HB],
                            in_=lnt[:M, :],
                        )
                        m0 += M
                    last_bulk_outs[t].append(
                        nc.gpsimd.dma_start(
                            out=newq[t, :, h * HB : (h + 1) * HB], in_=bank[:]
                        )
                    )

            # queue update: one dynamic-offset window DMA per bank,
            # overwriting the just-written bulk copy in DRAM.  Cores with
            # no update in a bank write original columns back (windows
            # always hold post-update content).  Emitted at the stream
            # tail on SP's HWDGE ring so their waits never head-block the
            # load or store streams.
            if "dyn" not in DEBUG_DISABLE:
                for t in range(T):
                    off = nc.values_load(
                        oft[0:1, t : t + 1],
                        min_val=0,
                        max_val=SH - S,
                        skip_runtime_bounds_check=True,
                    )
                    wdma = nc.sync.dma_start(
                        out=newq[t, :, ds(off, S)],
                        in_=kst[:, t * S : (t + 1) * S],
                    )
                    for b in last_bulk_outs[t][-NH:]:
                        tile.add_dep_helper(
                            wdma.ins, b.ins, reason="queue window after bulk"
                        )
    nc.compile()
    return nc


def _prepare(q, k, targets, queues, ptrs):
    """Host-side input prep: class grouping + scatter run construction."""
    perm = np.argsort(targets, kind="stable")
    tsorted = targets[perm]
    counts = np.bincount(targets, minlength=T)
    row_start = np.zeros(T + 1, np.int64)
    row_start[1:] = np.cumsum(counts)

    # positions: for class t the sorted samples go to ptrs[t], ptrs[t]+1, ...
    pos_sorted = np.empty(N, np.int64)
    for t in range(T):
        a, b = int(row_start[t]), int(row_start[t + 1])
        pos_sorted[a:b] = (int(ptrs[t]) + np.arange(b - a)) % K

    k_sorted = np.ascontiguousarray(k[perm])  # [N, C]

    # per (core, bank): the contiguous run of updates landing in this
    # core's shard.  (j0 = column offset within the shard, L, u0)
    runs = [[[] for _ in range(T)] for _ in range(NCORES)]
    for t in range(T):
        a, b = int(row_start[t]), int(row_start[t + 1])
        u = a
        while u < b:
            p = int(pos_sorted[u])
            c = p // SH
            # run end: consecutive positions within the same shard
            end = u + 1
            while (
                end < b
                and int(pos_sorted[end]) == int(pos_sorted[end - 1]) + 1
                and int(pos_sorted[end]) // SH == c
            ):
                end += 1
            runs[c][t].append((p - c * SH, end - u, u))
            u = end

    max_runs = max(len(runs[c][t]) for c in range(NCORES) for t in range(T))
    assert max_runs <= 1, f"expected <=1 run per (core, bank), got {max_runs}"
    S = max(
        [L for c in range(NCORES) for t in range(T) for (_, L, _) in runs[c][t]]
        + [1]
    )
    S = min(max(S, 16), SH)  # small floor reduces recompiles across inputs

    ks_host = np.empty((NCORES, C, T * S), np.float32)
    off_host = np.zeros((NCORES, 1, T), np.int32)
    for c in range(NCORES):
        qslab = queues[:, :, c * SH : (c + 1) * SH]
        for t in range(T):
            if runs[c][t]:
                j0, L, u0 = runs[c][t][0]
                off = max(0, min(j0, SH - S))
            else:
                j0, L, u0, off = 0, 0, 0, 0  # no-op window
            # window = post-update content of shard columns [off, off+S)
            w = np.array(qslab[t, :, off : off + S])
            if L:
                w[:, j0 - off : j0 - off + L] = k_sorted[u0 : u0 + L].T
            ks_host[c, :, t * S : (t + 1) * S] = w
            off_host[c, 0, t] = off
    return perm, counts, S, ks_host, off_host


def kernel(**inputs):
    q = np.ascontiguousarray(np.asarray(inputs["q"], dtype=np.float32))
    k = np.ascontiguousarray(np.asarray(inputs["k"], dtype=np.float32))
    targets = np.asarray(inputs["targets"], dtype=np.int32)
    queues = np.ascontiguousarray(np.asarray(inputs["queues"], dtype=np.float32))
    ptrs = np.asarray(inputs["ptrs"], dtype=np.int32)

    perm, counts, S, ks_host, off_host = _prepare(q, k, targets, queues, ptrs)

    key = (tuple(int(x) for x in counts), S)
    if key not in _cache:
        _cache[key] = _build(counts, S)
    nc = _cache[key]

    qT_sorted = np.ascontiguousarray(q[perm].T)
    cnts_in = counts.astype(np.int32).reshape(1, T)
    ptrs_in = np.ascontiguousarray(ptrs.reshape(1, T))

    in_maps = []
    for c in range(NCORES):
        in_maps.append(
            {
                "qt": qT_sorted,
                "qrow": q,
                "krow": k,
                "qsh": np.ascontiguousarray(queues[:, :, c * SH : (c + 1) * SH]),
                "ks": np.ascontiguousarray(ks_host[c]),
                "offt": off_host[c],
                "ptrs": ptrs_in,
                "cnts": cnts_in,
            }
        )
    res = run_bass_kernel_spmd(nc, in_maps, core_ids=list(range(NCORES)))
    results = res.results

    logits = np.empty((N, K + 1), np.float32)
    logits[:, 0] = results[0]["lpos"].T.reshape(N)
    for c in range(NCORES):
        logits[perm, 1 + c * SH : 1 + (c + 1) * SH] = results[c]["lneg"]
    new_queues = np.empty((T, C, K), np.float32)
    for c in range(NCORES):
        new_queues[:, :, c * SH : (c + 1) * SH] = results[c]["newq"]
    labels = np.zeros(N, np.int32)
    new_ptrs = results[0]["nptr"].reshape(T).astype(np.int32)
    return logits, labels, new_queues, new_ptrs


# revision 39
# speedup vs baseline: 5.5341x; 4.9618x over previous
"""MoCo forward + queue update kernel for Trainium2 (8 NeuronCores, SPMD).

Problem (nn_MoCo): given q,k [512,128] f32, targets [512] i32 (7 classes),
queues [7,128,65536] f32, ptrs [7] i32, compute
  l_pos[n]  = q[n].k[n]
  l_neg[n,:] = q[n] @ queues[targets[n]]
  logits    = concat([l_pos, l_neg], 1) / 0.07
  labels    = zeros
  new_queues = queues with column (t, ptrs[t]+rank) <- k[n] for each sample
  new_ptrs  = (ptrs + counts) % K

Sharding: split K=65536 into 8 shards of 8192 columns, one per core.  Each
core loads its shard of every queue bank once into SBUF, uses it as matmul
rhs (samples grouped by class so each bank is multiplied only by its own
class's queries -- 7x less FLOPs than the dense all-classes form), streams
the shard back out as new_queues, and stores its l_neg columns.  The
per-class scatter positions form one consecutive run per (core, bank)
(ring-buffer enqueue), so the queue update is a single dynamic-offset
[128, S] window DMA per bank written over the bulk copy at the stream
tail; cores without an update in a bank rewrite original columns (windows
always hold post-update content), keeping the SPMD program identical
across cores.  No collectives; the host concatenates the shards.
"""

import numpy as np

import concourse.mybir as mybir
import concourse.tile as tile
from concourse import bacc
from concourse.bass import ds
from concourse.bass_utils import run_bass_kernel_spmd

N, C, K, T = 512, 128, 65536, 7
TEMP = 0.07
NCORES = 8
SH = K // NCORES  # 8192 columns per core
CHUNK = 512  # matmul free dim (fp32 max)
HB = 4096  # pipeline tile width (half bank)
NH = SH // HB  # tiles per bank
USE_F16 = False  # cast matmul operands to fp16 (4x faster PE, ~1e-3 relerr)

f32 = mybir.dt.float32
i32 = mybir.dt.int32

_cache: dict = {}
DEBUG_DISABLE: set = set()  # dev-only: {"dyn", "mm", "lpos", "nptr"}


def _build(counts, S, rep=1):
    """Build the per-core Bass program (shared SPMD across all 8 cores).

    counts: per-class sample counts (compile-time constants).
    S: width of each dynamic-update window (max run length).
    rep: repeat the main pipeline (timing-harness use only).
    """
    counts = [int(x) for x in counts]
    row_start = [0] * (T + 1)
    for t in range(T):
        row_start[t + 1] = row_start[t] + counts[t]
    inv_temp = float(1.0 / np.float32(TEMP))

    nc = bacc.Bacc(None, target_bir_lowering=False)
    qT = nc.dram_tensor("qt", [C, N], f32, kind="ExternalInput")
    qrow = nc.dram_tensor("qrow", [N, C], f32, kind="ExternalInput")
    krow = nc.dram_tensor("krow", [N, C], f32, kind="ExternalInput")
    qsh = nc.dram_tensor("qsh", [T, C, SH], f32, kind="ExternalInput")
    ks = nc.dram_tensor("ks", [C, T * S], f32, kind="ExternalInput")
    offt = nc.dram_tensor("offt", [1, T], i32, kind="ExternalInput")
    ptrs = nc.dram_tensor("ptrs", [1, T], i32, kind="ExternalInput")
    cnts = nc.dram_tensor("cnts", [1, T], i32, kind="ExternalInput")
    lneg = nc.dram_tensor("lneg", [N, SH], f32, kind="ExternalOutput")
    newq = nc.dram_tensor("newq", [T, C, SH], f32, kind="ExternalOutput")
    lpos = nc.dram_tensor("lpos", [C, N // C], f32, kind="ExternalOutput")
    nptr = nc.dram_tensor("nptr", [1, T], i32, kind="ExternalOutput")

    with tile.TileContext(nc) as tc:
        with (
            tc.tile_pool(name="singles", bufs=1) as singles,
            tc.tile_pool(name="banks", bufs=6) as banks,
            tc.tile_pool(name="lnegs", bufs=4) as lnegs,
            tc.tile_pool(name="psum", bufs=8, space="PSUM") as psum,
        ):
            # --- queries, pre-scaled by 1/TEMP so PSUM holds scaled logits
            mmdt = mybir.dt.float16 if USE_F16 else f32
            qt_raw = singles.tile([C, N], f32)
            nc.sync.dma_start(out=qt_raw[:], in_=qT[:, :])
            qts = singles.tile([C, N], mmdt)
            nc.vector.tensor_scalar_mul(qts[:], qt_raw[:], inv_temp)

            # --- l_pos: per-sample q.k via elementwise mul + free-dim reduce
            B = N // C  # 4 row-blocks of 128
            if "lpos" not in DEBUG_DISABLE:
                qp = singles.tile([C, B, C], f32)
                nc.sync.dma_start(
                    out=qp[:], in_=qrow[:, :].rearrange("(b p) c -> p b c", p=C)
                )
                kp = singles.tile([C, B, C], f32)
                nc.sync.dma_start(
                    out=kp[:], in_=krow[:, :].rearrange("(b p) c -> p b c", p=C)
                )
                prod = singles.tile([C, B, C], f32)
                nc.vector.tensor_mul(prod[:], qp[:], kp[:])
                lp = singles.tile([C, B], f32)
                nc.vector.reduce_sum(lp[:], prod[:], axis=mybir.AxisListType.X)
                lps = singles.tile([C, B], f32)
                nc.scalar.mul(lps[:], lp[:], inv_temp)
                nc.sync.dma_start(out=lpos[:, :], in_=lps[:])

            # --- new_ptrs = (ptrs + counts) % K  (sum < 2K so mod = cond-sub)
            if "nptr" not in DEBUG_DISABLE:
                pt = singles.tile([1, T], i32)
                nc.sync.dma_start(out=pt[:], in_=ptrs[:, :])
                ct = singles.tile([1, T], i32)
                nc.sync.dma_start(out=ct[:], in_=cnts[:, :])
                s1 = singles.tile([1, T], i32)
                nc.vector.tensor_add(s1[:], pt[:], ct[:])
                mge = singles.tile([1, T], i32)
                nc.vector.tensor_scalar(
                    mge[:], s1[:], K, None, op0=mybir.AluOpType.is_ge
                )
                mk = singles.tile([1, T], i32)
                nc.vector.tensor_scalar_mul(mk[:], mge[:], K)
                s2 = singles.tile([1, T], i32)
                nc.vector.tensor_tensor(
                    s2[:], s1[:], mk[:], op=mybir.AluOpType.subtract
                )
                nc.sync.dma_start(out=nptr[:, :], in_=s2[:])

            # --- scatter sources: per bank a [C, S] column window holding
            # the post-update content, written straight to DRAM after the
            # bulk copy (so the bulk store never waits on the update)
            kst = singles.tile([C, T * S], f32)
            nc.sync.dma_start(out=kst[:], in_=ks[:, :])
            oft = singles.tile([1, T], i32)
            nc.sync.dma_start(out=oft[:], in_=offt[:, :])

            # --- main pipeline: per (bank, half-bank) tile
            last_bulk_outs = {t: [] for t in range(T)}  # final-rep bulk outs
            for t in [t for _ in range(rep) for t in range(T)]:
                for h in range(NH):
                    bank = banks.tile([C, HB], f32)
                    nc.sync.dma_start(
                        out=bank[:], in_=qsh[t, :, h * HB : (h + 1) * HB]
                    )
                    if USE_F16:
                        bank_mm = banks.tile([C, HB], mmdt, tag="bank_mm")
                        nc.gpsimd.tensor_copy(out=bank_mm[:], in_=bank[:])
                    else:
                        bank_mm = bank
                    # l_neg rows for class t (row groups of <=128)
                    m0 = 0
                    if "mm" in DEBUG_DISABLE:
                        m0 = counts[t]
                    while m0 < counts[t]:
                        M = min(128, counts[t] - m0)
                        r0 = row_start[t] + m0
                        lnt = lnegs.tile([C, HB], f32)
                        for j in range(HB // CHUNK):
                            ps = psum.tile([128, CHUNK], f32, space="PSUM")
                            nc.tensor.matmul(
                                ps[:M],
                                lhsT=qts[:, r0 : r0 + M],
                                rhs=bank_mm[:, j * CHUNK : (j + 1) * CHUNK],
                                start=True,
                                stop=True,
                            )
                            # split PSUM->SBUF copies across ACT and DVE
                            if j % 2 == 0:
                                nc.scalar.copy(
                                    lnt[:M, j * CHUNK : (j + 1) * CHUNK], ps[:M]
                                )
                            else:
                                nc.vector.tensor_copy(
                                    out=lnt[:M, j * CHUNK : (j + 1) * CHUNK],
                                    in_=ps[:M],
                                )
                        nc.gpsimd.dma_start(
                            out=lneg[r0 : r0 + M, h * HB : (h + 1) * HB],
                            in_=lnt[:M, :],
                        )
                        m0 += M
                    last_bulk_outs[t].append(
                        nc.gpsimd.dma_start(
                            out=newq[t, :, h * HB : (h + 1) * HB], in_=bank[:]
                        )
                    )

            # queue update: one dynamic-offset window DMA per bank,
            # overwriting the just-written bulk copy in DRAM.  Cores with
            # no update in a bank write original columns back (windows
            # always hold post-update content).  Emitted at the stream
            # tail on SP's HWDGE ring so their waits never head-block the
            # load or store streams.
            if "dyn" not in DEBUG_DISABLE:
                for t in range(T):
                    off = nc.values_load(
                        oft[0:1, t : t + 1],
                        min_val=0,
                        max_val=SH - S,
                        skip_runtime_bounds_check=True,
                    )
                    wdma = nc.sync.dma_start(
                        out=newq[t, :, ds(off, S)],
                        in_=kst[:, t * S : (t + 1) * S],
                    )
                    for b in last_bulk_outs[t][-NH:]:
                        tile.add_dep_helper(
                            wdma.ins, b.ins, reason="queue window after bulk"
                        )
    nc.compile()
    return nc


def _prepare(q, k, targets, queues, ptrs):
    """Host-side input prep: class grouping + scatter run construction."""
    perm = np.argsort(targets, kind="stable")
    tsorted = targets[perm]
    counts = np.bincount(targets, minlength=T)
    row_start = np.zeros(T + 1, np.int64)
    row_start[1:] = np.cumsum(counts)

    # positions: for class t the sorted samples go to ptrs[t], ptrs[t]+1, ...
    pos_sorted = np.empty(N, np.int64)
    for t in range(T):
        a, b = int(row_start[t]), int(row_start[t + 1])
        pos_sorted[a:b] = (int(ptrs[t]) + np.arange(b - a)) % K

    k_sorted = np.ascontiguousarray(k[perm])  # [N, C]

    # per (core, bank): the contiguous run of updates landing in this
    # core's shard.  (j0 = column offset within the shard, L, u0)
    runs = [[[] for _ in range(T)] for _ in range(NCORES)]
    for t in range(T):
        a, b = int(row_start[t]), int(row_start[t + 1])
        u = a
        while u < b:
            p = int(pos_sorted[u])
            c = p // SH
            # run end: consecutive positions within the same shard
            end = u + 1
            while (
                end < b
                and int(pos_sorted[end]) == int(pos_sorted[end - 1]) + 1
                and int(pos_sorted[end]) // SH == c
            ):
                end += 1
            runs[c][t].append((p - c * SH, end - u, u))
            u = end

    max_runs = max(len(runs[c][t]) for c in range(NCORES) for t in range(T))
    assert max_runs <= 1, f"expected <=1 run per (core, bank), got {max_runs}"
    S = max(
        [L for c in range(NCORES) for t in range(T) for (_, L, _) in runs[c][t]]
        + [1]
    )
    S = min(max(S, 16), SH)  # small floor reduces recompiles across inputs

    ks_host = np.empty((NCORES, C, T * S), np.float32)
    off_host = np.zeros((NCORES, 1, T), np.int32)
    for c in range(NCORES):
        qslab = queues[:, :, c * SH : (c + 1) * SH]
        for t in range(T):
            if runs[c][t]:
                j0, L, u0 = runs[c][t][0]
                off = max(0, min(j0, SH - S))
            else:
                j0, L, u0, off = 0, 0, 0, 0  # no-op window
            # window = post-update content of shard columns [off, off+S)
            w = np.array(qslab[t, :, off : off + S])
            if L:
                w[:, j0 - off : j0 - off + L] = k_sorted[u0 : u0 + L].T
            ks_host[c, :, t * S : (t + 1) * S] = w
            off_host[c, 0, t] = off
    return perm, counts, S, ks_host, off_host


def kernel(**inputs):
    q = np.ascontiguousarray(np.asarray(inputs["q"], dtype=np.float32))
    k = np.ascontiguousarray(np.asarray(inputs["k"], dtype=np.float32))
    targets = np.asarray(inputs["targets"], dtype=np.int32)
    queues = np.ascontiguousarray(np.asarray(inputs["queues"], dtype=np.float32))
    ptrs = np.asarray(inputs["ptrs"], dtype=np.int32)

    perm, counts, S, ks_host, off_host = _prepare(q, k, targets, queues, ptrs)

    key = (tuple(int(x) for x in counts), S)
    if key not in _cache:
        _cache[key] = _build(counts, S)
    nc = _cache[key]

    qT_sorted = np.ascontiguousarray(q[perm].T)
    cnts_in = counts.astype(np.int32).reshape(1, T)
    ptrs_in = np.ascontiguousarray(ptrs.reshape(1, T))

    in_maps = []
    for c in range(NCORES):
        in_maps.append(
            {
                "qt": qT_sorted,
                "qrow": q,
                "krow": k,
                "qsh": np.ascontiguousarray(queues[:, :, c * SH : (c + 1) * SH]),
                "ks": np.ascontiguousarray(ks_host[c]),
                "offt": off_host[c],
                "ptrs": ptrs_in,
                "cnts": cnts_in,
            }
        )
    res = run_bass_kernel_spmd(nc, in_maps, core_ids=list(range(NCORES)))
    results = res.results

    logits = np.empty((N, K + 1), np.float32)
    logits[:, 0] = results[0]["lpos"].T.reshape(N)
    for c in range(NCORES):
        logits[perm, 1 + c * SH : 1 + (c + 1) * SH] = results[c]["lneg"]
    new_queues = np.empty((T, C, K), np.float32)
    for c in range(NCORES):
        new_queues[:, :, c * SH : (c + 1) * SH] = results[c]["newq"]
    labels = np.zeros(N, np.int32)
    new_ptrs = results[0]["nptr"].reshape(T).astype(np.int32)
    return logits, labels, new_queues, new_ptrs
